# revision 1
# baseline (speedup 1.0000x reference)
"""MoE routing kernel (nn_BrainModel_1640677507517) for 8x TRN2 NeuronCores.

Math (per token x[10]):
  h1 = relu(x @ W1[e] + b1[e])          e=0..7, 32 units   -> 256 feats
  h2 = relu(h1 @ W2[e] + b2[e])                            -> 256 feats
  outs = tanh(h2 @ W3[e] + b3[e])                          -> 8x2
  g = relu(x @ G1 + gb1); w = softmax(g @ G2 + gb2)        -> 8
  fused = sum_e outs[e]*w[e]                               -> 2
  y = tanh(relu([fused,x] @ R1 + rb1) @ R2 + rb2)          -> 2

Layout: feature-major (features on SBUF partitions, tokens on the free dim),
float32r matmul operands (1 cycle/row on the PE vs 4 for plain fp32).
The narrow softmax/refine tail is batched over 4 token-tiles at 32-aligned
partition blocks so elementwise ops run with ~full lane occupancy.
Host pre-transposes x -> x_t [10, B_shard]; output returns as y_t
[2, B_shard] and is transposed back on the host. Pure data parallel over
8 cores; weights replicated.
"""

import numpy as np

import concourse.bacc as bacc
import concourse.tile as tile
from concourse import mybir
from concourse.bass_utils import run_bass_kernel_spmd

N_CORES = 8
B = 1048576
BS = B // N_CORES  # 131072 tokens per core
NT = 512           # tokens per tile
NTILES = BS // NT  # 256 (multiple of 4)

F32 = mybir.dt.float32
FR = mybir.dt.float32r
AF = mybir.ActivationFunctionType

# --- weight-pack column layout (wc: [128, CC] float32r) ---
_off = 0


def _col(n):
    global _off
    o = _off
    _off += n
    return o


C_W1A = _col(128)   # [10,128]   W1 experts 0-3
C_W1B = _col(128)   # [10,128]   W1 experts 4-7
C_G1 = _col(64)     # [10,64]
C_W2A = _col(128)   # [128,128]  block-diag W2 experts 0-3
C_W2B = _col(128)   # [128,128]  block-diag W2 experts 4-7
C_G1X = [_col(128) for _ in range(2)]   # [10,128] G1 at cols 64q, rest 0
C_W3X = [_col(128) for _ in range(8)]   # [128,128] W3 A/B per tile j at
#                                         cols 32j (zero elsewhere)
C_G2DD = [_col(128) for _ in range(2)]  # [128,128] dup G2 block-diag, pair p
C_NUMS = _col(98)   # [128,98]   numerator selectors (4 tiles)
C_DENS = _col(98)   # [128,98]   denominator selectors (4 tiles)
C_R1X = [_col(128) for _ in range(2)]   # [10,128] R1 x-part at cols 64q
C_R1FA = _col(128)  # [98,128]   R1 rows 0:2 for pair0 (tiles 0,1 of batch)
C_R1FB = _col(128)  # [98,128]   R1 rows 0:2 for pair1 (tiles 2,3)
C_R2BD = [_col(36) for _ in range(2)]   # [128,36] R2 block-diag at cols 32p
CC = ((_off + 31) // 32) * 32

# --- bias pack (wb: [128, CB] float32) ---
C_B1A, C_B1B, C_B2A, C_B2B, C_GB1X2, C_B3O4, C_GB2D4, C_GB2P4, C_RB1X2, \
    C_RB24 = range(10)
CB = 16

TRACE = False
LAST_RESULTS = None


def _pack_consts(W1, b1, W2, b2, W3, b3, G1, gb1, G2, gb2, R1, rb1, R2, rb2):
    wc = np.zeros((128, CC), dtype=np.float32)
    w1 = np.transpose(W1, (1, 0, 2)).reshape(10, 256)
    wc[0:10, C_W1A:C_W1A + 128] = w1[:, 0:128]
    wc[0:10, C_W1B:C_W1B + 128] = w1[:, 128:256]
    wc[0:10, C_G1:C_G1 + 64] = G1
    for q in range(2):
        wc[0:10, C_G1X[q] + 64 * q:C_G1X[q] + 64 * q + 64] = G1
    for e in range(4):
        wc[e * 32:(e + 1) * 32, C_W2A + e * 32:C_W2A + (e + 1) * 32] = W2[e]
        wc[e * 32:(e + 1) * 32, C_W2B + e * 32:C_W2B + (e + 1) * 32] = W2[e + 4]
    # outs block row r (within a 32-row tile block): r = 2e+a (e<4),
    # r = 8+2(e-4)+a (e>=4); all other columns zero (the 8 W3 matmuls
    # accumulate into psO4 [128, NT], each contributing its own block)
    for j in range(4):
        for e in range(4):
            wc[e * 32:(e + 1) * 32,
               C_W3X[2 * j] + 32 * j + 2 * e:
               C_W3X[2 * j] + 32 * j + 2 * e + 2] = W3[e]
            wc[e * 32:(e + 1) * 32,
               C_W3X[2 * j + 1] + 32 * j + 8 + 2 * e:
               C_W3X[2 * j + 1] + 32 * j + 8 + 2 * e + 2] = W3[e + 4]
    # G2 pair block-diags over g2 [128, NT] (even tile rows 0:64, odd 64:128):
    # dup: out block col c<16 -> G2[:, c//2]; plain: col c<8 -> G2[:, c]
    for p in range(2):
        for c in range(16):
            wc[0:64, C_G2DD[p] + 64 * p + c] = G2[:, c // 2]
            wc[64:128, C_G2DD[p] + 64 * p + 32 + c] = G2[:, c // 2]
    # numerator selector: col 32j+a sums ewp rows 32j+r (r<16, r%2==a)
    for j in range(4):
        for r in range(16):
            wc[32 * j + r, C_NUMS + 32 * j + (r % 2)] = 1.0
    # denominator selector: every col of block j sums the 16 dup'd exp rows
    # of ew4 with coefficient 0.5 (each expert appears twice); identical
    # cols keep psD finite everywhere for the reciprocal
    for j in range(4):
        ncols = min(32, 98 - 32 * j)
        for c in range(ncols):
            wc[32 * j:32 * j + 16, C_DENS + 32 * j + c] = 0.5
    for q in range(2):
        wc[0:10, C_R1X[q] + 64 * q:C_R1X[q] + 64 * q + 64] = R1[2:12]
    # fused part of R1: fused4 rows 0:2/32:34/64:66/96:98 are tiles 0..3
    wc[0:2, C_R1FA:C_R1FA + 64] = R1[0:2]
    wc[32:34, C_R1FA + 64:C_R1FA + 128] = R1[0:2]
    wc[64:66, C_R1FB:C_R1FB + 64] = R1[0:2]
    wc[96:98, C_R1FB + 64:C_R1FB + 128] = R1[0:2]
    for p in range(2):
        wc[0:64, C_R2BD[p] + 32 * p:C_R2BD[p] + 32 * p + 2] = R2
        wc[64:128, C_R2BD[p] + 32 * p + 2:C_R2BD[p] + 32 * p + 4] = R2

    wb = np.zeros((128, CB), dtype=np.float32)
    wb[0:128, C_B1A] = b1[0:4].reshape(-1)
    wb[0:128, C_B1B] = b1[4:8].reshape(-1)
    wb[0:128, C_B2A] = b2[0:4].reshape(-1)
    wb[0:128, C_B2B] = b2[4:8].reshape(-1)
    wb[0:64, C_GB1X2] = gb1
    wb[64:128, C_GB1X2] = gb1
    b3f = b3.reshape(-1)
    for j in range(4):
        wb[32 * j:32 * j + 16, C_B3O4] = b3f
        wb[32 * j:32 * j + 16, C_GB2D4] = np.repeat(gb2, 2)
        wb[32 * j:32 * j + 8, C_GB2P4] = gb2
    wb[0:64, C_RB1X2] = rb1
    wb[64:128, C_RB1X2] = rb1
    for rr in (0, 2, 32, 34):
        wb[rr:rr + 2, C_RB24] = rb2
    return wc, wb


def _build_bass():
    nc = bacc.Bacc("TRN2", debug=False, enable_asserts=False, num_devices=N_CORES)
    x_t = nc.dram_tensor("x_t", [10, BS], FR, kind="ExternalInput").ap()
    wc = nc.dram_tensor("wc", [128, CC], FR, kind="ExternalInput").ap()
    wb = nc.dram_tensor("wb", [128, CB], F32, kind="ExternalInput").ap()
    y_t = nc.dram_tensor("y_t", [2, BS], F32, kind="ExternalOutput").ap()
    ADD, MAX = mybir.AluOpType.add, mybir.AluOpType.max

    with tile.TileContext(nc) as tc:
        with (
            tc.tile_pool(name="const", bufs=1) as cp,
            tc.tile_pool(name="io", bufs=12) as iop,
            tc.tile_pool(name="act", bufs=6) as ap_,
            tc.tile_pool(name="ps", bufs=1, space="PSUM") as pp,
        ):
            C = cp.tile([128, CC], FR)
            nc.sync.dma_start(C[:, :], wc[:, :])
            Cb = cp.tile([128, CB], F32)
            nc.sync.dma_start(Cb[:, :], wb[:, :])
            # PE warm-up consuming the const DMA so steady-state matmuls
            # carry at most one semaphore wait
            pwm = pp.tile([1, 8], F32, tag="bt", bufs=3)
            nc.tensor.matmul(pwm[:, :], C[0:1, 0:1], C[0:1, 0:8],
                             start=True, stop=True)

            def head(b, part_cb=None):
                """Per-4-tile-batch: input DMAs, expert MLP layers, gating,
                logits; returns carried state for tail(). part_cb(k) is
                invoked between pairs to interleave the previous batch's
                tail work."""
                xcs = []
                psO4 = pp.tile([128, NT], F32, tag="bt", bufs=3)
                psW4 = pp.tile([128, NT], F32, tag="bt", bufs=3)
                for p in range(2):  # pair index
                    psg = pp.tile([128, NT], F32, tag="psg", bufs=1)
                    for q in range(2):
                        j = 2 * p + q
                        if part_cb is not None:
                            part_cb(j)
                        i = 4 * b + j
                        sl = slice(i * NT, (i + 1) * NT)
                        xc = iop.tile([10, NT], FR, tag="xc")
                        nc.sync.dma_start(xc[0:10, :], x_t[:, sl])
                        xcs.append(xc)

                        ps1a = pp.tile([128, NT], F32, tag="psL1", bufs=2)
                        nc.tensor.matmul(ps1a[:, :], C[0:10, C_W1A:C_W1A + 128],
                                         xc[0:10, :], start=True, stop=True)
                        ps1b = pp.tile([128, NT], F32, tag="psL1", bufs=2)
                        nc.tensor.matmul(ps1b[:, :], C[0:10, C_W1B:C_W1B + 128],
                                         xc[0:10, :], start=True, stop=True)
                        nc.tensor.matmul(psg[:, :],
                                         C[0:10, C_G1X[q]:C_G1X[q] + 128],
                                         xc[0:10, :], start=(q == 0),
                                         stop=(q == 1))

                        h1a = ap_.tile([128, NT], FR, tag="h1a")
                        nc.scalar.activation(h1a[:, :], ps1a[:, :], AF.Relu,
                                             bias=Cb[0:128, C_B1A:C_B1A + 1])
                        h1b = ap_.tile([128, NT], FR, tag="h1b")
                        nc.vector.tensor_scalar(h1b[:, :], ps1b[:, :],
                                                Cb[0:128, C_B1B:C_B1B + 1], 0.0,
                                                ADD, MAX)

                        ps2a = pp.tile([128, NT], F32, tag="psL2", bufs=2)
                        nc.tensor.matmul(ps2a[:, :], C[:, C_W2A:C_W2A + 128],
                                         h1a[:, :], start=True, stop=True)
                        ps2b = pp.tile([128, NT], F32, tag="psL2", bufs=2)
                        nc.tensor.matmul(ps2b[:, :], C[:, C_W2B:C_W2B + 128],
                                         h1b[:, :], start=True, stop=True)
                        h2a = ap_.tile([128, NT], FR, tag="h2a")
                        nc.scalar.activation(h2a[:, :], ps2a[:, :], AF.Relu,
                                             bias=Cb[0:128, C_B2A:C_B2A + 1])
                        h2b = ap_.tile([128, NT], FR, tag="h2b")
                        nc.vector.tensor_scalar(h2b[:, :], ps2b[:, :],
                                                Cb[0:128, C_B2B:C_B2B + 1], 0.0,
                                                ADD, MAX)

                        # outs block j accumulates into psO4 (8 matmuls)
                        ca = C_W3X[2 * j]
                        cb = C_W3X[2 * j + 1]
                        nc.tensor.matmul(psO4[:, :], C[:, ca:ca + 128],
                                         h2a[:, :], start=(j == 0),
                                         stop=False)
                        nc.tensor.matmul(psO4[:, :], C[:, cb:cb + 128],
                                         h2b[:, :], start=False,
                                         stop=(j == 3))

                    # gating pair: relu then dup/plain logit matmuls
                    g2 = ap_.tile([128, NT], FR, tag="g2")
                    nc.scalar.activation(g2[:, :], psg[:, :], AF.Relu,
                                         bias=Cb[0:128, C_GB1X2:C_GB1X2 + 1])
                    cd = C_G2DD[p]
                    nc.tensor.matmul(psW4[:, :], C[:, cd:cd + 128], g2[:, :],
                                     start=(p == 0), stop=(p == 1))

                outs4 = ap_.tile([128, NT], FR, tag="outs4")
                nc.scalar.activation(outs4[:, :], psO4[:, :], AF.Tanh,
                                     bias=Cb[0:128, C_B3O4:C_B3O4 + 1])
                ew4 = ap_.tile([128, NT], FR, tag="ew4")
                nc.scalar.activation(ew4[:, :], psW4[:, :], AF.Exp,
                                     bias=Cb[0:128, C_GB2D4:C_GB2D4 + 1])
                return b, xcs, outs4, ew4

            def tail_a0(st):
                b, xcs, outs4, ew4 = st
                ewp4 = ap_.tile([128, NT], FR, tag="ewp4")
                nc.vector.tensor_mul(ewp4[:, :], ew4[:, :], outs4[:, :])
                psR4 = pp.tile([98, NT], F32, tag="bt", bufs=3)
                nc.tensor.matmul(psR4[:, :], C[:, C_NUMS:C_NUMS + 98],
                                 ewp4[:, :], start=True, stop=True)
                psD4 = pp.tile([98, NT], F32, tag="bt", bufs=3)
                nc.tensor.matmul(psD4[:, :], C[:, C_DENS:C_DENS + 98],
                                 ew4[:, :], start=True, stop=True)
                return psR4, psD4

            def tail_a1(ps):
                psR4, psD4 = ps
                rcp4 = ap_.tile([98, NT], F32, tag="rcp4")
                nc.vector.reciprocal(rcp4[:, :], psD4[:, :])
                fused4 = ap_.tile([98, NT], FR, tag="fused4")
                nc.vector.tensor_mul(fused4[:, :], psR4[:, :], rcp4[:, :])
                return fused4

            def tail_b(st, fused4, psY4, prange):
                b, xcs, outs4, ew4 = st
                for p in prange:
                    psr1 = pp.tile([128, NT], F32, tag="psg", bufs=1)
                    for q in range(2):
                        cr = C_R1X[q]
                        nc.tensor.matmul(psr1[:, :], C[0:10, cr:cr + 128],
                                         xcs[2 * p + q][0:10, :],
                                         start=(q == 0), stop=False)
                    c_r1f = C_R1FA if p == 0 else C_R1FB
                    nc.tensor.matmul(psr1[:, :], C[0:98, c_r1f:c_r1f + 128],
                                     fused4[:, :], start=False, stop=True)
                    r2 = ap_.tile([128, NT], FR, tag="r2")
                    nc.vector.tensor_scalar(r2[:, :], psr1[:, :],
                                            Cb[0:128, C_RB1X2:C_RB1X2 + 1],
                                            0.0, ADD, MAX)
                    c2 = C_R2BD[p]
                    nc.tensor.matmul(psY4[:, :], C[:, c2:c2 + 36], r2[:, :],
                                     start=(p == 0), stop=(p == 1))
            def tail_c(st, psY4):
                b = st[0]
                yt4 = ap_.tile([36, NT], F32, tag="yt4")
                nc.scalar.activation(yt4[:, :], psY4[:, :], AF.Tanh,
                                     bias=Cb[0:36, C_RB24:C_RB24 + 1])
                for j in range(4):
                    i = 4 * b + j
                    sl = slice(i * NT, (i + 1) * NT)
                    rr = (0, 2, 32, 34)[j]
                    nc.sync.dma_start(y_t[:, sl], yt4[rr:rr + 2, :])

            carried = None
            for b in range(NTILES // 4):
                state = {}

                def cb(j, _c=carried, _s=state):
                    if _c is None:
                        return
                    if j == 1:
                        _s['ps'] = tail_a0(_c)
                    elif j == 2:
                        _s['f'] = tail_a1(_s['ps'])
                        _s['y'] = pp.tile([36, NT], F32, tag="bt", bufs=3, name="psY4")
                    elif j == 3:
                        tail_b(_c, _s['f'], _s['y'], (0,))
                st = head(b, cb)
                if carried is not None:
                    tail_b(carried, state['f'], state['y'], (1,))
                    tail_c(carried, state['y'])
                carried = st
            ps = tail_a0(carried)
            f = tail_a1(ps)
            y4 = pp.tile([36, NT], F32, tag="bt", bufs=3, name="psY4e")
            tail_b(carried, f, y4, (0, 1))
            tail_c(carried, y4)
    nc.compile()
    return nc


_NC_CACHE = None


def kernel(x, W1, b1, W2, b2, W3, b3, G1, gb1, G2, gb2, R1, rb1, R2, rb2):
    global _NC_CACHE, LAST_RESULTS
    x = np.asarray(x)
    wc, wb = _pack_consts(np.asarray(W1), np.asarray(b1), np.asarray(W2),
                          np.asarray(b2), np.asarray(W3), np.asarray(b3),
                          np.asarray(G1), np.asarray(gb1), np.asarray(G2),
                          np.asarray(gb2), np.asarray(R1), np.asarray(rb1),
                          np.asarray(R2), np.asarray(rb2))
    if _NC_CACHE is None:
        _NC_CACHE = _build_bass()
    nc = _NC_CACHE
    in_maps = []
    for c in range(N_CORES):
        xs = np.ascontiguousarray(x[c * BS:(c + 1) * BS].T)
        in_maps.append({"x_t": xs, "wc": wc, "wb": wb})
    res = run_bass_kernel_spmd(nc, in_maps, core_ids=list(range(N_CORES)),
                               trace=TRACE)
    LAST_RESULTS = res
    y = np.concatenate([res.results[c]["y_t"].T for c in range(N_CORES)], axis=0)
    return y



# revision 2
# speedup vs baseline: 2.2406x; 2.2406x over previous
"""MoE routing kernel (nn_BrainModel_1640677507517) for 8x TRN2 NeuronCores.

Math (per token x[10]):
  h1 = relu(x @ W1[e] + b1[e])          e=0..7, 32 units   -> 256 feats
  h2 = relu(h1 @ W2[e] + b2[e])                            -> 256 feats
  outs = tanh(h2 @ W3[e] + b3[e])                          -> 8x2
  g = relu(x @ G1 + gb1); w = softmax(g @ G2 + gb2)        -> 8
  fused = sum_e outs[e]*w[e]                               -> 2
  y = tanh(relu([fused,x] @ R1 + rb1) @ R2 + rb2)          -> 2

Layout: feature-major (features on SBUF partitions, tokens on the free dim),
float32r matmul operands (1 cycle/row on the PE vs 4 for plain fp32).
The narrow softmax/refine tail is batched over 4 token-tiles at 32-aligned
partition blocks so elementwise ops run with ~full lane occupancy.
Host pre-transposes x -> x_t [10, B_shard]; output returns as y_t
[2, B_shard] and is transposed back on the host. Pure data parallel over
8 cores; weights replicated.
"""

import numpy as np

import concourse.bacc as bacc
import concourse.tile as tile
from concourse import mybir
from concourse.bass_utils import run_bass_kernel_spmd

N_CORES = 8
B = 1048576
BS = B // N_CORES  # 131072 tokens per core
NT = 512           # tokens per tile
NTILES = BS // NT  # 256 (multiple of 4)

F32 = mybir.dt.float32
FR = mybir.dt.float32r
AF = mybir.ActivationFunctionType

# --- weight-pack column layout (wc: [128, CC] float32r) ---
_off = 0


def _col(n):
    global _off
    o = _off
    _off += n
    return o


C_W1A = _col(128)   # [10,128]   W1 experts 0-3
C_W1B = _col(128)   # [10,128]   W1 experts 4-7
C_G1 = _col(64)     # [10,64]
C_W2A = _col(128)   # [128,128]  block-diag W2 experts 0-3
C_W2B = _col(128)   # [128,128]  block-diag W2 experts 4-7
C_G1X = [_col(128) for _ in range(2)]   # [10,128] G1 at cols 64q, rest 0
C_W3X = [_col(128) for _ in range(8)]   # [128,128] W3 A/B per tile j at
#                                         cols 32j (zero elsewhere)
C_G2DD = [_col(128) for _ in range(2)]  # [128,128] dup G2 block-diag, pair p
C_NUMS = _col(98)   # [128,98]   numerator selectors (4 tiles)
C_DENS = _col(98)   # [128,98]   denominator selectors (4 tiles)
C_R1X = [_col(128) for _ in range(2)]   # [10,128] R1 x-part at cols 64q
C_R1FA = _col(128)  # [98,128]   R1 rows 0:2 for pair0 (tiles 0,1 of batch)
C_R1FB = _col(128)  # [98,128]   R1 rows 0:2 for pair1 (tiles 2,3)
C_R2BD = [_col(36) for _ in range(2)]   # [128,36] R2 block-diag at cols 32p
CC = ((_off + 31) // 32) * 32

# --- bias pack (wb: [128, CB] float32) ---
C_B1A, C_B1B, C_B2A, C_B2B, C_GB1X2, C_B3O4, C_GB2D4, C_GB2P4, C_RB1X2, \
    C_RB24 = range(10)
CB = 16

TRACE = False
LAST_RESULTS = None


def _pack_consts(W1, b1, W2, b2, W3, b3, G1, gb1, G2, gb2, R1, rb1, R2, rb2):
    wc = np.zeros((128, CC), dtype=np.float32)
    w1 = np.transpose(W1, (1, 0, 2)).reshape(10, 256)
    wc[0:10, C_W1A:C_W1A + 128] = w1[:, 0:128]
    wc[0:10, C_W1B:C_W1B + 128] = w1[:, 128:256]
    wc[0:10, C_G1:C_G1 + 64] = G1
    for q in range(2):
        wc[0:10, C_G1X[q] + 64 * q:C_G1X[q] + 64 * q + 64] = G1
    for e in range(4):
        wc[e * 32:(e + 1) * 32, C_W2A + e * 32:C_W2A + (e + 1) * 32] = W2[e]
        wc[e * 32:(e + 1) * 32, C_W2B + e * 32:C_W2B + (e + 1) * 32] = W2[e + 4]
    # outs block row r (within a 32-row tile block): r = 2e+a (e<4),
    # r = 8+2(e-4)+a (e>=4); all other columns zero (the 8 W3 matmuls
    # accumulate into psO4 [128, NT], each contributing its own block)
    for j in range(4):
        for e in range(4):
            wc[e * 32:(e + 1) * 32,
               C_W3X[2 * j] + 32 * j + 2 * e:
               C_W3X[2 * j] + 32 * j + 2 * e + 2] = W3[e]
            wc[e * 32:(e + 1) * 32,
               C_W3X[2 * j + 1] + 32 * j + 8 + 2 * e:
               C_W3X[2 * j + 1] + 32 * j + 8 + 2 * e + 2] = W3[e + 4]
    # G2 pair block-diags over g2 [128, NT] (even tile rows 0:64, odd 64:128):
    # dup: out block col c<16 -> G2[:, c//2]; plain: col c<8 -> G2[:, c]
    for p in range(2):
        for c in range(16):
            wc[0:64, C_G2DD[p] + 64 * p + c] = G2[:, c // 2]
            wc[64:128, C_G2DD[p] + 64 * p + 32 + c] = G2[:, c // 2]
    # numerator selector: col 32j+a sums ewp rows 32j+r (r<16, r%2==a)
    for j in range(4):
        for r in range(16):
            wc[32 * j + r, C_NUMS + 32 * j + (r % 2)] = 1.0
    # denominator selector: every col of block j sums the 16 dup'd exp rows
    # of ew4 with coefficient 0.5 (each expert appears twice); identical
    # cols keep psD finite everywhere for the reciprocal
    for j in range(4):
        ncols = min(32, 98 - 32 * j)
        for c in range(ncols):
            wc[32 * j:32 * j + 16, C_DENS + 32 * j + c] = 0.5
    for q in range(2):
        wc[0:10, C_R1X[q] + 64 * q:C_R1X[q] + 64 * q + 64] = R1[2:12]
    # fused part of R1: fused4 rows 0:2/32:34/64:66/96:98 are tiles 0..3
    wc[0:2, C_R1FA:C_R1FA + 64] = R1[0:2]
    wc[32:34, C_R1FA + 64:C_R1FA + 128] = R1[0:2]
    wc[64:66, C_R1FB:C_R1FB + 64] = R1[0:2]
    wc[96:98, C_R1FB + 64:C_R1FB + 128] = R1[0:2]
    for p in range(2):
        wc[0:64, C_R2BD[p] + 32 * p:C_R2BD[p] + 32 * p + 2] = R2
        wc[64:128, C_R2BD[p] + 32 * p + 2:C_R2BD[p] + 32 * p + 4] = R2

    wb = np.zeros((128, CB), dtype=np.float32)
    wb[0:128, C_B1A] = b1[0:4].reshape(-1)
    wb[0:128, C_B1B] = b1[4:8].reshape(-1)
    wb[0:128, C_B2A] = b2[0:4].reshape(-1)
    wb[0:128, C_B2B] = b2[4:8].reshape(-1)
    wb[0:64, C_GB1X2] = gb1
    wb[64:128, C_GB1X2] = gb1
    b3f = b3.reshape(-1)
    for j in range(4):
        wb[32 * j:32 * j + 16, C_B3O4] = b3f
        wb[32 * j:32 * j + 16, C_GB2D4] = np.repeat(gb2, 2)
        wb[32 * j:32 * j + 8, C_GB2P4] = gb2
    wb[0:64, C_RB1X2] = rb1
    wb[64:128, C_RB1X2] = rb1
    for rr in (0, 2, 32, 34):
        wb[rr:rr + 2, C_RB24] = rb2
    return wc, wb


def _build_bass():
    nc = bacc.Bacc("TRN2", debug=False, enable_asserts=False, num_devices=N_CORES)
    x_t = nc.dram_tensor("x_t", [10, BS], FR, kind="ExternalInput").ap()
    wc = nc.dram_tensor("wc", [128, CC], FR, kind="ExternalInput").ap()
    wb = nc.dram_tensor("wb", [128, CB], F32, kind="ExternalInput").ap()
    y_t = nc.dram_tensor("y_t", [2, BS], F32, kind="ExternalOutput").ap()
    ADD, MAX = mybir.AluOpType.add, mybir.AluOpType.max

    with tile.TileContext(nc) as tc:
        with (
            tc.tile_pool(name="const", bufs=1) as cp,
            tc.tile_pool(name="io", bufs=12) as iop,
            tc.tile_pool(name="act", bufs=6) as ap_,
            tc.tile_pool(name="ps", bufs=1, space="PSUM") as pp,
        ):
            C = cp.tile([128, CC], FR)
            nc.sync.dma_start(C[:, :], wc[:, :])
            Cb = cp.tile([128, CB], F32)
            nc.sync.dma_start(Cb[:, :], wb[:, :])
            # PE warm-up consuming the const DMA so steady-state matmuls
            # carry at most one semaphore wait
            pwm = pp.tile([1, 8], F32, tag="bt", bufs=3)
            nc.tensor.matmul(pwm[:, :], C[0:1, 0:1], C[0:1, 0:8],
                             start=True, stop=True)

            def head(b, part_cb=None):
                """Per-4-tile-batch: input DMAs, expert MLP layers, gating,
                logits; returns carried state for tail(). part_cb(k) is
                invoked between pairs to interleave the previous batch's
                tail work."""
                xcs = []
                psO4 = pp.tile([128, NT], F32, tag="bt", bufs=3)
                psW4 = pp.tile([128, NT], F32, tag="bt", bufs=3)
                for p in range(2):  # pair index
                    psg = pp.tile([128, NT], F32, tag="psg", bufs=1)
                    for q in range(2):
                        j = 2 * p + q
                        if part_cb is not None:
                            part_cb(j)
                        i = 4 * b + j
                        sl = slice(i * NT, (i + 1) * NT)
                        xc = iop.tile([10, NT], FR, tag="xc")
                        nc.sync.dma_start(xc[0:10, :], x_t[:, sl])
                        xcs.append(xc)

                        ps1a = pp.tile([128, NT], F32, tag="psL1", bufs=2)
                        nc.tensor.matmul(ps1a[:, :], C[0:10, C_W1A:C_W1A + 128],
                                         xc[0:10, :], start=True, stop=True)
                        ps1b = pp.tile([128, NT], F32, tag="psL1", bufs=2)
                        nc.tensor.matmul(ps1b[:, :], C[0:10, C_W1B:C_W1B + 128],
                                         xc[0:10, :], start=True, stop=True)
                        nc.tensor.matmul(psg[:, :],
                                         C[0:10, C_G1X[q]:C_G1X[q] + 128],
                                         xc[0:10, :], start=(q == 0),
                                         stop=(q == 1))

                        h1a = ap_.tile([128, NT], FR, tag="h1a")
                        nc.scalar.activation(h1a[:, :], ps1a[:, :], AF.Relu,
                                             bias=Cb[0:128, C_B1A:C_B1A + 1])
                        h1b = ap_.tile([128, NT], FR, tag="h1b")
                        nc.vector.tensor_scalar(h1b[:, :], ps1b[:, :],
                                                Cb[0:128, C_B1B:C_B1B + 1], 0.0,
                                                ADD, MAX)

                        ps2a = pp.tile([128, NT], F32, tag="psL2", bufs=2)
                        nc.tensor.matmul(ps2a[:, :], C[:, C_W2A:C_W2A + 128],
                                         h1a[:, :], start=True, stop=True)
                        ps2b = pp.tile([128, NT], F32, tag="psL2", bufs=2)
                        nc.tensor.matmul(ps2b[:, :], C[:, C_W2B:C_W2B + 128],
                                         h1b[:, :], start=True, stop=True)
                        h2a = ap_.tile([128, NT], FR, tag="h2a")
                        nc.scalar.activation(h2a[:, :], ps2a[:, :], AF.Relu,
                                             bias=Cb[0:128, C_B2A:C_B2A + 1])
                        h2b = ap_.tile([128, NT], FR, tag="h2b")
                        nc.vector.tensor_scalar(h2b[:, :], ps2b[:, :],
                                                Cb[0:128, C_B2B:C_B2B + 1], 0.0,
                                                ADD, MAX)

                        # outs block j accumulates into psO4 (8 matmuls)
                        ca = C_W3X[2 * j]
                        cb = C_W3X[2 * j + 1]
                        nc.tensor.matmul(psO4[:, :], C[:, ca:ca + 128],
                                         h2a[:, :], start=(j == 0),
                                         stop=False)
                        nc.tensor.matmul(psO4[:, :], C[:, cb:cb + 128],
                                         h2b[:, :], start=False,
                                         stop=(j == 3))

                    # gating pair: relu then dup/plain logit matmuls
                    g2 = ap_.tile([128, NT], FR, tag="g2")
                    nc.scalar.activation(g2[:, :], psg[:, :], AF.Relu,
                                         bias=Cb[0:128, C_GB1X2:C_GB1X2 + 1])
                    cd = C_G2DD[p]
                    nc.tensor.matmul(psW4[:, :], C[:, cd:cd + 128], g2[:, :],
                                     start=(p == 0), stop=(p == 1))

                outs4 = ap_.tile([128, NT], FR, tag="outs4")
                nc.scalar.activation(outs4[:, :], psO4[:, :], AF.Tanh,
                                     bias=Cb[0:128, C_B3O4:C_B3O4 + 1])
                ew4 = ap_.tile([128, NT], FR, tag="ew4")
                nc.scalar.activation(ew4[:, :], psW4[:, :], AF.Exp,
                                     bias=Cb[0:128, C_GB2D4:C_GB2D4 + 1])
                return b, xcs, outs4, ew4

            def tail_a0(st):
                b, xcs, outs4, ew4 = st
                ewp4 = ap_.tile([128, NT], FR, tag="ewp4")
                nc.vector.tensor_mul(ewp4[:, :], ew4[:, :], outs4[:, :])
                psR4 = pp.tile([98, NT], F32, tag="bt", bufs=3)
                nc.tensor.matmul(psR4[:, :], C[:, C_NUMS:C_NUMS + 98],
                                 ewp4[:, :], start=True, stop=True)
                psD4 = pp.tile([98, NT], F32, tag="bt", bufs=3)
                nc.tensor.matmul(psD4[:, :], C[:, C_DENS:C_DENS + 98],
                                 ew4[:, :], start=True, stop=True)
                return psR4, psD4

            def tail_a1(ps):
                psR4, psD4 = ps
                rcp4 = ap_.tile([98, NT], F32, tag="rcp4")
                nc.vector.reciprocal_approx_fast(rcp4[:, :], psD4[:, :])
                fused4 = ap_.tile([98, NT], FR, tag="fused4")
                nc.vector.tensor_mul(fused4[:, :], psR4[:, :], rcp4[:, :])
                return fused4

            def tail_b(st, fused4, psY4, prange):
                b, xcs, outs4, ew4 = st
                for p in prange:
                    psr1 = pp.tile([128, NT], F32, tag="psg", bufs=1)
                    for q in range(2):
                        cr = C_R1X[q]
                        nc.tensor.matmul(psr1[:, :], C[0:10, cr:cr + 128],
                                         xcs[2 * p + q][0:10, :],
                                         start=(q == 0), stop=False)
                    c_r1f = C_R1FA if p == 0 else C_R1FB
                    nc.tensor.matmul(psr1[:, :], C[0:98, c_r1f:c_r1f + 128],
                                     fused4[:, :], start=False, stop=True)
                    r2 = ap_.tile([128, NT], FR, tag="r2")
                    nc.vector.tensor_scalar(r2[:, :], psr1[:, :],
                                            Cb[0:128, C_RB1X2:C_RB1X2 + 1],
                                            0.0, ADD, MAX)
                    c2 = C_R2BD[p]
                    nc.tensor.matmul(psY4[:, :], C[:, c2:c2 + 36], r2[:, :],
                                     start=(p == 0), stop=(p == 1))
            def tail_c(st, psY4):
                b = st[0]
                yt4 = ap_.tile([36, NT], F32, tag="yt4")
                nc.scalar.activation(yt4[:, :], psY4[:, :], AF.Tanh,
                                     bias=Cb[0:36, C_RB24:C_RB24 + 1])
                for j in range(4):
                    i = 4 * b + j
                    sl = slice(i * NT, (i + 1) * NT)
                    rr = (0, 2, 32, 34)[j]
                    nc.sync.dma_start(y_t[:, sl], yt4[rr:rr + 2, :])

            carried = None
            for b in range(NTILES // 4):
                state = {}

                def cb(j, _c=carried, _s=state):
                    if _c is None:
                        return
                    if j == 1:
                        _s['ps'] = tail_a0(_c)
                    elif j == 2:
                        _s['f'] = tail_a1(_s['ps'])
                        _s['y'] = pp.tile([36, NT], F32, tag="bt", bufs=3, name="psY4")
                    elif j == 3:
                        tail_b(_c, _s['f'], _s['y'], (0,))
                st = head(b, cb)
                if carried is not None:
                    tail_b(carried, state['f'], state['y'], (1,))
                    tail_c(carried, state['y'])
                carried = st
            ps = tail_a0(carried)
            f = tail_a1(ps)
            y4 = pp.tile([36, NT], F32, tag="bt", bufs=3, name="psY4e")
            tail_b(carried, f, y4, (0, 1))
            tail_c(carried, y4)
    nc.compile()
    return nc


_NC_CACHE = None


def kernel(x, W1, b1, W2, b2, W3, b3, G1, gb1, G2, gb2, R1, rb1, R2, rb2):
    global _NC_CACHE, LAST_RESULTS
    x = np.asarray(x)
    wc, wb = _pack_consts(np.asarray(W1), np.asarray(b1), np.asarray(W2),
                          np.asarray(b2), np.asarray(W3), np.asarray(b3),
                          np.asarray(G1), np.asarray(gb1), np.asarray(G2),
                          np.asarray(gb2), np.asarray(R1), np.asarray(rb1),
                          np.asarray(R2), np.asarray(rb2))
    if _NC_CACHE is None:
        _NC_CACHE = _build_bass()
    nc = _NC_CACHE
    in_maps = []
    for c in range(N_CORES):
        xs = np.ascontiguousarray(x[c * BS:(c + 1) * BS].T)
        in_maps.append({"x_t": xs, "wc": wc, "wb": wb})
    res = run_bass_kernel_spmd(nc, in_maps, core_ids=list(range(N_CORES)),
                               trace=TRACE)
    LAST_RESULTS = res
    y = np.concatenate([res.results[c]["y_t"].T for c in range(N_CORES)], axis=0)
    return y

